# revision 8
# baseline (speedup 1.0000x reference)
"""Chamfer loss kernel for Trainium2 (8 NeuronCores).

Problem: predictions [4, 8192, 3], targets [4, 8192, 3] (f32).
loss = mean_b mean_n min_m |x_bn - y_bm|^2  +  mean_b mean_m min_n |x_bn - y_bm|^2

Strategy
--------
* 8 cores = 4 batches x 2 halves of the prediction points. Core c handles
  batch c//2 with x-half c%2 (4096 x-points) against the full 8192 y-points.
* Distance matrix via one K=5 augmented matmul on the PE:
      u_n = [x0, x1, x2, |x|^2, 1],  v_m = [-2y0, -2y1, -2y2, 1, |y|^2]
      d2[n, m] = u_n . v_m
  The augmentation (a [5, N] f32 tensor per side) is prepared on host and
  shipped as ONE tensor (single DMA -> single semaphore wait; PE LDWEIGHTS
  only has one sync-wait slot).
* Per row-tile i (128 x-points): 16 matmuls of N=512 into PSUM ([128, 2048]
  f32 x 4 groups). ScalarE extracts each PSUM group into an SBUF fp16 tile
  (fp16 rounding is monotone, so min of rounded = rounded min; ~1e-4-level
  relative error on the final loss).
* VectorE: col-min accumulate (tensor_tensor min, fp16 SBUF -> 2x mode) and
  row-min over all 8192 cols via tensor_scalar(max(.,0)) with fused
  min-reduction (fp16 SBUF -> 4x mode).
* The col accumulator [128, 8192] fp16 is DMA'd out raw; the final 128-way
  cross-partition min + means happen on host (trivial numpy work).
"""

import numpy as np

_NC_CACHE = None

_B = 4          # batches
_N = 8192       # points per cloud
_H = 4096       # x-points per core (half batch)
_NCORES = 8
_NI = _H // 128      # 32 row tiles
_GRP = 2048          # y-points per PSUM group (4 banks)
_NG = _N // _GRP     # 4 groups
_MM_N = 512          # matmul free dim (one PSUM bank, f32)


def _build_nc(compile_module=True):
    import concourse.bacc as bacc
    import concourse.mybir as mybir
    from concourse.tile import TileContext

    f32 = mybir.dt.float32
    f16 = mybir.dt.float16
    Alu = mybir.AluOpType

    # Bacc (not plain Bass): its compile() pass moves multi-waits off
    # matmuls (PE LDWEIGHTS has a single sync-wait slot in the ISA).
    nc = bacc.Bacc()
    uv = nc.dram_tensor("uv", [5, _H + _N], f32, kind="ExternalInput")
    # out_x[p, i] = min_m d2(x_{i*128+p}, y_m), clamped at 0
    out_x = nc.dram_tensor("out_x", [128, _NI], f32, kind="ExternalOutput")
    # out_y[p, m] = min over {x_{i*128+p}, all i} of d2(x, y_m)  (fp16, unclamped)
    out_y = nc.dram_tensor("out_y", [128, _N], f16, kind="ExternalOutput")

    with TileContext(nc) as tc:
        with (
            tc.tile_pool(name="const", bufs=1) as cpool,
            tc.tile_pool(name="work", bufs=2) as wpool,
            tc.tile_pool(name="psum", bufs=2, space="PSUM") as ppool,
        ):
            uv_sb = cpool.tile([5, _H + _N], f32)
            nc.sync.dma_start(uv_sb[:], uv[:])
            u_sb = uv_sb[:, :_H]
            v_sb = uv_sb[:, _H:]

            # col-min accumulator over row tiles
            colacc = cpool.tile([128, _N], f16)
            nc.vector.memset(colacc[:], 65504.0)  # fp16 max; d2 <= ~200

            rowmin = cpool.tile([128, _NI], f32)
            scr = cpool.tile([128, _N], f16)  # discarded main output of row op

            for i in range(_NI):
                s = wpool.tile([128, _N], f16, tag="s")
                lhsT = u_sb[:, i * 128 : (i + 1) * 128]
                for g in range(_NG):
                    ps = ppool.tile([128, _GRP], f32, tag="mm")
                    for k in range(_GRP // _MM_N):
                        c0 = g * _GRP + k * _MM_N
                        nc.tensor.matmul(
                            ps[:, k * _MM_N : (k + 1) * _MM_N],
                            lhsT,
                            v_sb[:, c0 : c0 + _MM_N],
                            start=True,
                            stop=True,
                        )
                    gsl = slice(g * _GRP, (g + 1) * _GRP)
                    # PSUM f32 -> SBUF fp16 extraction on ScalarE
                    nc.scalar.copy(s[:, gsl], ps[:])
                    # col-min accumulate (DVE, fp16 2x mode)
                    nc.vector.tensor_tensor(
                        colacc[:, gsl], s[:, gsl], colacc[:, gsl], Alu.min
                    )
                # row-min over all 8192 cols, fused relu (DVE, fp16 4x mode)
                nc.vector.tensor_scalar(
                    scr[:],
                    s[:],
                    0.0,
                    None,
                    Alu.max,
                    Alu.min,
                    accum_out=rowmin[:, i : i + 1],
                )

            nc.sync.dma_start(out_x[:], rowmin[:])
            nc.sync.dma_start(out_y[:], colacc[:])
    if compile_module:
        nc.finalize()  # Bacc.finalize runs compile() (wait legalization etc.)
    return nc


def _get_nc():
    global _NC_CACHE
    if _NC_CACHE is None:
        _NC_CACHE = _build_nc()
    return _NC_CACHE


def _make_in_maps(predictions, targets):
    in_maps = []
    for c in range(_NCORES):
        b, h = divmod(c, 2)
        x = np.asarray(predictions[b, h * _H : (h + 1) * _H], dtype=np.float32)
        y = np.asarray(targets[b], dtype=np.float32)
        uv = np.empty((5, _H + _N), np.float32)
        uv[0:3, :_H] = x.T
        uv[3, :_H] = (x * x).sum(axis=-1)
        uv[4, :_H] = 1.0
        uv[0:3, _H:] = -2.0 * y.T
        uv[3, _H:] = 1.0
        uv[4, _H:] = (y * y).sum(axis=-1)
        in_maps.append({"uv": uv})
    return in_maps


def _combine(results):
    loss = 0.0
    for b in range(_B):
        r0, r1 = results[2 * b], results[2 * b + 1]
        cx = np.concatenate(
            [
                np.ascontiguousarray(r0["out_x"].T).ravel(),
                np.ascontiguousarray(r1["out_x"].T).ravel(),
            ]
        )
        cy = np.minimum(
            r0["out_y"].astype(np.float32).min(axis=0),
            r1["out_y"].astype(np.float32).min(axis=0),
        )
        cx = np.maximum(cx, 0.0)
        cy = np.maximum(cy, 0.0)
        loss += cx.mean(dtype=np.float64) + cy.mean(dtype=np.float64)
    loss /= _B
    return np.array(loss, dtype=np.float32)


def kernel(predictions, targets):
    nc = _get_nc()
    in_maps = _make_in_maps(predictions, targets)
    try:
        from concourse.bass_utils import run_bass_kernel_spmd

        res = run_bass_kernel_spmd(nc, in_maps, core_ids=list(range(_NCORES)))
        results = res.results
    except ModuleNotFoundError:
        # axon NTFF trace hook unavailable (BASS_TRACE set in env) — run
        # the execute step directly via PJRT.
        from concourse import bass2jax

        results = bass2jax.run_bass_via_pjrt(nc, in_maps, n_cores=_NCORES)
    return _combine(results)


# revision 11
# speedup vs baseline: 179.5442x; 179.5442x over previous
"""Chamfer loss kernel for Trainium2 (8 NeuronCores).

Problem: predictions [4, 8192, 3], targets [4, 8192, 3] (f32).
loss = mean_b mean_n min_m |x_bn - y_bm|^2  +  mean_b mean_m min_n |x_bn - y_bm|^2

Strategy
--------
* 8 cores = 4 batches x 2 halves of the prediction points. Core c handles
  batch c//2 with x-half c%2 (4096 x-points) against the full 8192 y-points.
* Distance matrix on the PE via an augmented matmul:
      u_n = [x0, x1, x2, |x|^2, 1],  v_m = [-2y0, -2y1, -2y2, 1, |y|^2]
      d2[n, m] = u_n . v_m
  FP32 moving operands stream at 1/4 rate on the PE, so u and v are split
  on host into bf16 hi+lo pairs and the 4 cross products are stacked into
  a single K=20 bf16 matmul (exact (u_hi+u_lo).(v_hi+v_lo), f32 accum):
      lhsT rows = [u_hi; u_lo; u_hi; u_lo]   (20 x 4096)
      rhs  rows = [v_hi; v_hi; v_lo; v_lo]   (20 x 8192)
  Inputs are effectively rounded to ~2^-17 relative — far below the fp16
  rounding applied at extraction. Shipped as ONE tensor (single DMA ->
  single semaphore wait; PE LDWEIGHTS has one sync-wait slot).
* Per row-tile i (128 x-points): 16 matmuls of N=512 into PSUM ([128, 2048]
  f32 x 4 groups). ScalarE extracts each PSUM group into an SBUF fp16 tile
  (fp16 rounding is monotone, so min of rounded = rounded min).
* VectorE: col-min accumulate (tensor_tensor min, fp16 SBUF -> 2x mode) and
  row-min over all 8192 cols via tensor_scalar(max(.,0)) with fused
  min-reduction (fp16 SBUF -> 4x mode).
* Tail: PE-transpose of the col accumulator 128x128 blocks + DVE min-reduce
  -> cross-partition col-min on chip. Host combines halves, clamps, means.
"""

import numpy as np

_NC_CACHE = None

_B = 4          # batches
_N = 8192       # points per cloud
_H = 4096       # x-points per core (half batch)
_NCORES = 8
_NI = _H // 128      # 32 row tiles
_GRP = 2048          # y-points per PSUM group (4 banks)
_NG = _N // _GRP     # 4 groups
_MM_N = 512          # matmul free dim (one PSUM bank, f32)
_K = 20              # 4 bf16 cross-product blocks of 5


def _build_nc(compile_module=True, loop_repeats=None):
    import concourse.bacc as bacc
    import concourse.mybir as mybir
    from concourse import masks
    from concourse.tile import TileContext

    f32 = mybir.dt.float32
    f16 = mybir.dt.float16
    bf16 = mybir.dt.bfloat16
    Alu = mybir.AluOpType

    # Bacc (not plain Bass): its compile() pass moves multi-waits off
    # matmuls (PE LDWEIGHTS has a single sync-wait slot in the ISA).
    nc = bacc.Bacc()
    uv = nc.dram_tensor("uv", [_K, _H + _N], bf16, kind="ExternalInput")
    # out_x[p, i] = min_m d2(x_{i*128+p}, y_m), clamped at 0
    out_x = nc.dram_tensor("out_x", [128, _NI], f32, kind="ExternalOutput")
    # out_y[c, blk] = min over this core's x of d2(x, y_{blk*128+c}) (unclamped)
    out_y = nc.dram_tensor("out_y", [128, _N // 128], f32, kind="ExternalOutput")

    with TileContext(nc) as tc:
        with (
            tc.tile_pool(name="const", bufs=1) as cpool,
            tc.tile_pool(name="work", bufs=2) as wpool,
            tc.tile_pool(name="psum", bufs=2, space="PSUM") as ppool,
        ):
            uv_sb = cpool.tile([_K, _H + _N], bf16)
            nc.sync.dma_start(uv_sb[:], uv[:])
            u_sb = uv_sb[:, :_H]
            v_sb = uv_sb[:, _H:]

            ident = cpool.tile([128, 128], f16)
            masks.make_identity(nc, ident[:])

            # col-min accumulator over row tiles
            colacc = cpool.tile([128, _N], f16)
            nc.vector.memset(colacc[:], 65504.0)  # fp16 max; d2 <= ~200

            rowmin = cpool.tile([128, _NI], f32)
            colmin = cpool.tile([128, _N // 128], f32)
            scr = cpool.tile([128, _N], f16)  # discarded main output of row op

            def main_block(_iv=None):
                for i in range(_NI):
                    s = wpool.tile([128, _N], f16, tag="s", name="s")
                    lhsT = u_sb[:, i * 128 : (i + 1) * 128]
                    for g in range(_NG):
                        ps = ppool.tile([128, _GRP], f32, tag="mm", name="ps")
                        for k in range(_GRP // _MM_N):
                            c0 = g * _GRP + k * _MM_N
                            nc.tensor.matmul(
                                ps[:, k * _MM_N : (k + 1) * _MM_N],
                                lhsT,
                                v_sb[:, c0 : c0 + _MM_N],
                                start=True,
                                stop=True,
                            )
                        gsl = slice(g * _GRP, (g + 1) * _GRP)
                        # PSUM f32 -> SBUF fp16 extraction on ScalarE
                        nc.scalar.copy(s[:, gsl], ps[:])
                        # col-min accumulate (DVE, fp16 2x mode)
                        nc.vector.tensor_tensor(
                            colacc[:, gsl], s[:, gsl], colacc[:, gsl], Alu.min
                        )
                    # row-min over 8192 cols, fused relu (DVE, fp16 4x mode)
                    nc.vector.tensor_scalar(
                        scr[:],
                        s[:],
                        0.0,
                        None,
                        Alu.max,
                        Alu.min,
                        accum_out=rowmin[:, i : i + 1],
                    )

            if loop_repeats is None:
                main_block()
            else:
                # Timing mode: re-run the identical compute loop_repeats
                # times (same result; colacc/rowmin are idempotent).
                with tc.For_i(0, loop_repeats, 1) as iv:
                    main_block(iv)

            # Cross-partition min of colacc: PE-transpose 128x128 blocks
            # (4 per PSUM tile), then DVE min-reduce the inner dim.
            nblk = _N // 128  # 64
            for t in range(nblk // 4):
                tp = ppool.tile([128, 512], f16, tag="mm", name="tp")
                for k in range(4):
                    blk = t * 4 + k
                    nc.tensor.transpose(
                        tp[:, k * 128 : (k + 1) * 128],
                        colacc[:, blk * 128 : (blk + 1) * 128],
                        ident[:],
                    )
                nc.vector.tensor_reduce(
                    colmin[:, t * 4 : (t + 1) * 4],
                    tp.rearrange("p (b c) -> p b c", b=4),
                    axis=mybir.AxisListType.X,
                    op=Alu.min,
                )

            nc.sync.dma_start(out_x[:], rowmin[:])
            nc.sync.dma_start(out_y[:], colmin[:])
    if compile_module:
        nc.finalize()  # Bacc.finalize runs compile() (wait legalization etc.)
    return nc


def _get_nc():
    global _NC_CACHE
    if _NC_CACHE is None:
        _NC_CACHE = _build_nc()
    return _NC_CACHE


def _hi_lo(a):
    """Split f32 array into bf16 hi + lo with hi + lo ≈ a (~2^-17 rel)."""
    import ml_dtypes

    hi = a.astype(ml_dtypes.bfloat16)
    lo = (a - hi.astype(np.float32)).astype(ml_dtypes.bfloat16)
    return hi, lo


def _make_in_maps(predictions, targets):
    import ml_dtypes

    bf16 = ml_dtypes.bfloat16
    in_maps = []
    for c in range(_NCORES):
        b, h = divmod(c, 2)
        x = np.asarray(predictions[b, h * _H : (h + 1) * _H], dtype=np.float32)
        y = np.asarray(targets[b], dtype=np.float32)
        u = np.empty((5, _H), np.float32)
        u[0:3] = x.T
        u[3] = (x * x).sum(axis=-1)
        u[4] = 1.0
        v = np.empty((5, _N), np.float32)
        v[0:3] = -2.0 * y.T
        v[3] = 1.0
        v[4] = (y * y).sum(axis=-1)
        u_hi, u_lo = _hi_lo(u)
        v_hi, v_lo = _hi_lo(v)
        uv = np.empty((_K, _H + _N), bf16)
        uv[0:5, :_H] = u_hi
        uv[5:10, :_H] = u_lo
        uv[10:15, :_H] = u_hi
        uv[15:20, :_H] = u_lo
        uv[0:5, _H:] = v_hi
        uv[5:10, _H:] = v_hi
        uv[10:15, _H:] = v_lo
        uv[15:20, _H:] = v_lo
        in_maps.append({"uv": uv})
    return in_maps


def _combine(results):
    loss = 0.0
    for b in range(_B):
        r0, r1 = results[2 * b], results[2 * b + 1]
        cx = np.concatenate(
            [
                np.ascontiguousarray(r0["out_x"].T).ravel(),
                np.ascontiguousarray(r1["out_x"].T).ravel(),
            ]
        )
        cy = np.minimum(
            np.ascontiguousarray(r0["out_y"].T).ravel(),
            np.ascontiguousarray(r1["out_y"].T).ravel(),
        )
        cx = np.maximum(cx, 0.0)
        cy = np.maximum(cy, 0.0)
        loss += cx.mean(dtype=np.float64) + cy.mean(dtype=np.float64)
    loss /= _B
    return np.array(loss, dtype=np.float32)


def kernel(predictions, targets):
    nc = _get_nc()
    in_maps = _make_in_maps(predictions, targets)
    try:
        from concourse.bass_utils import run_bass_kernel_spmd

        res = run_bass_kernel_spmd(nc, in_maps, core_ids=list(range(_NCORES)))
        results = res.results
    except ModuleNotFoundError:
        # axon NTFF trace hook unavailable (BASS_TRACE set in env) — run
        # the execute step directly via PJRT.
        from concourse import bass2jax

        results = bass2jax.run_bass_via_pjrt(nc, in_maps, n_cores=_NCORES)
    return _combine(results)


# revision 22
# speedup vs baseline: 195.2186x; 1.0873x over previous
"""Chamfer loss kernel for Trainium2 (8 NeuronCores).

Problem: predictions [4, 8192, 3], targets [4, 8192, 3] (f32).
loss = mean_b mean_n min_m |x_bn - y_bm|^2  +  mean_b mean_m min_n |x_bn - y_bm|^2

Strategy
--------
* 8 cores = 4 batches x 2 halves of the prediction points. Core c handles
  batch c//2 with x-half c%2 (4096 x-points) against the full 8192 y-points.
* Distance matrix on the PE via an augmented matmul:
      u_n = [x0, x1, x2, |x|^2, 1],  v_m = [-2y0, -2y1, -2y2, 1, |y|^2]
      d2[n, m] = u_n . v_m
  FP32 moving operands stream at 1/4 rate on the PE, so u and v are split
  on host into bf16 hi+lo pairs and the 4 cross products are stacked into
  a single K=20 bf16 matmul (exact (u_hi+u_lo).(v_hi+v_lo), f32 accum):
      lhsT rows = [u_hi; u_lo; u_hi; u_lo]   (20 x 4096)
      rhs  rows = [v_hi; v_hi; v_lo; v_lo]   (20 x 8192)
  Inputs are effectively rounded to ~2^-17 relative — far below the fp16
  rounding applied at extraction. Shipped as ONE tensor (single DMA ->
  single semaphore wait; PE LDWEIGHTS has one sync-wait slot).
* Per row-tile i (128 x-points): 16 matmuls of N=512 into PSUM ([128, 2048]
  f32 x 4 groups). ScalarE extracts each PSUM group into an SBUF fp16 tile
  (fp16 rounding is monotone, so min of rounded = rounded min).
* VectorE: col-min accumulate (tensor_tensor min, fp16 SBUF -> 2x mode) and
  row-min over all 8192 cols via tensor_scalar(max(.,0)) with fused
  min-reduction (fp16 SBUF -> 4x mode).
* Tail: PE-transpose of the col accumulator 128x128 blocks + DVE min-reduce
  -> cross-partition col-min on chip. Host combines halves, clamps, means.
"""

import numpy as np

_NC_CACHE = None

_B = 4          # batches
_N = 8192       # points per cloud
_H = 4096       # x-points per core (half batch)
_NCORES = 8
_NI = _H // 128      # 32 row tiles
_GRP = 2048          # y-points per PSUM group (4 banks)
_NG = _N // _GRP     # 4 groups
_MM_N = 512          # matmul free dim (one PSUM bank, f32)
_K = 20              # 4 bf16 cross-product blocks of 5


def _build_nc(compile_module=True, loop_repeats=None, parts=("mm", "act", "col", "row")):
    import concourse.bacc as bacc
    import concourse.mybir as mybir
    from concourse import masks
    from concourse.tile import TileContext

    f32 = mybir.dt.float32
    f16 = mybir.dt.float16
    bf16 = mybir.dt.bfloat16
    Alu = mybir.AluOpType

    # Bacc (not plain Bass): its compile() pass moves multi-waits off
    # matmuls (PE LDWEIGHTS has a single sync-wait slot in the ISA).
    nc = bacc.Bacc()
    uv = nc.dram_tensor("uv", [_K, _H + _N], bf16, kind="ExternalInput")
    # out_x[p, i] = min_m d2(x_{i*128+p}, y_m), clamped at 0
    out_x = nc.dram_tensor("out_x", [128, _NI], f32, kind="ExternalOutput")
    # out_y[c, blk] = min over this core's x of d2(x, y_{blk*128+c}) (unclamped)
    out_y = nc.dram_tensor("out_y", [128, _N // 128], f32, kind="ExternalOutput")

    with TileContext(nc) as tc:
        with (
            tc.tile_pool(name="const", bufs=1) as cpool,
            tc.tile_pool(name="work", bufs=3) as wpool,
            tc.tile_pool(name="psum", bufs=2, space="PSUM") as ppool,
        ):
            # Two copies of uv in SBUF partition bands 0-19 and 32-51: the
            # matmuls alternate PE row-groups (tile_position (0,0)/(32,0))
            # so each LDWEIGHTS overlaps the other group's in-flight MATMUL
            # (same-row-group LDW cannot be pulled ahead).
            uv_sb = cpool.tile([32 + _K, _H + _N], bf16)
            nc.sync.dma_start(uv_sb[:_K, :], uv[:])
            nc.sync.dma_start(uv_sb[32 : 32 + _K, :], uv[:])
            u_bands = (uv_sb[:_K, :_H], uv_sb[32 : 32 + _K, :_H])
            v_bands = (uv_sb[:_K, _H:], uv_sb[32 : 32 + _K, _H:])

            ident = cpool.tile([128, 128], f16)
            masks.make_identity(nc, ident[:])

            # col-min accumulator over row tiles
            colacc = cpool.tile([128, _N], f16)
            nc.vector.memset(colacc[:], 65504.0)  # fp16 max; d2 <= ~200

            rowmin = cpool.tile([128, _NI], f32)
            colmin = cpool.tile([128, _N // 128], f32)
            scr = cpool.tile([128, _N], f16)  # discarded main output of row op
            if "row" not in parts:
                nc.vector.memset(rowmin[:], 0.0)

            def main_block(_iv=None):
                mm_idx = 0
                for i in range(_NI):
                    s_dt = f32 if "actf32" in parts else f16
                    s = wpool.tile([128, _N], s_dt, tag="s", name="s")
                    if "act" not in parts and ("col" in parts or "row" in parts):
                        nc.vector.memset(s[:], 1.0)
                    for g in range(_NG):
                        gsl = slice(g * _GRP, (g + 1) * _GRP)
                        if "mm" in parts:
                            ps = ppool.tile([128, _GRP], f32, tag="mm", name="ps")
                            for k in range(_GRP // _MM_N):
                                c0 = g * _GRP + k * _MM_N
                                band = mm_idx % 2
                                mm_idx += 1
                                nc.tensor.matmul(
                                    ps[:, k * _MM_N : (k + 1) * _MM_N],
                                    u_bands[band][:, i * 128 : (i + 1) * 128],
                                    v_bands[band][:, c0 : c0 + _MM_N],
                                    start=True,
                                    stop=True,
                                    tile_position=(32 * band, 0),
                                )
                            if "act" in parts:
                                # PSUM f32 -> SBUF fp16 extraction on ScalarE
                                nc.scalar.copy(s[:, gsl], ps[:])
                        if "col" in parts:
                            # col-min accumulate (DVE, fp16 2x mode)
                            nc.vector.tensor_tensor(
                                colacc[:, gsl], s[:, gsl], colacc[:, gsl], Alu.min
                            )
                    if "colbig" in parts:
                        nc.vector.tensor_tensor(
                            colacc[:], s[:], colacc[:], Alu.min
                        )
                    if "row" in parts:
                        # row-min over 8192 cols, fused relu (DVE, fp16 4x)
                        nc.vector.tensor_scalar(
                            scr[:],
                            s[:],
                            0.0,
                            None,
                            Alu.max,
                            Alu.min,
                            accum_out=rowmin[:, i : i + 1],
                        )

            if loop_repeats is None:
                main_block()
            else:
                # Timing mode: re-run the identical compute loop_repeats
                # times (same result; colacc/rowmin are idempotent).
                with tc.For_i(0, loop_repeats, 1) as iv:
                    main_block(iv)

            # Cross-partition min of colacc: PE-transpose 128x128 blocks
            # (4 per PSUM tile), then DVE min-reduce the inner dim.
            nblk = _N // 128  # 64
            for t in range(nblk // 4):
                tp = ppool.tile([128, 512], f16, tag="mm", name="tp")
                for k in range(4):
                    blk = t * 4 + k
                    nc.tensor.transpose(
                        tp[:, k * 128 : (k + 1) * 128],
                        colacc[:, blk * 128 : (blk + 1) * 128],
                        ident[:],
                    )
                nc.vector.tensor_reduce(
                    colmin[:, t * 4 : (t + 1) * 4],
                    tp.rearrange("p (b c) -> p b c", b=4),
                    axis=mybir.AxisListType.X,
                    op=Alu.min,
                )

            nc.sync.dma_start(out_x[:], rowmin[:])
            nc.sync.dma_start(out_y[:], colmin[:])
    if compile_module:
        nc.finalize()  # Bacc.finalize runs compile() (wait legalization etc.)
    return nc


def _get_nc():
    global _NC_CACHE
    if _NC_CACHE is None:
        _NC_CACHE = _build_nc()
    return _NC_CACHE


def _hi_lo(a):
    """Split f32 array into bf16 hi + lo with hi + lo ≈ a (~2^-17 rel)."""
    import ml_dtypes

    hi = a.astype(ml_dtypes.bfloat16)
    lo = (a - hi.astype(np.float32)).astype(ml_dtypes.bfloat16)
    return hi, lo


def _make_in_maps(predictions, targets):
    import ml_dtypes

    bf16 = ml_dtypes.bfloat16
    in_maps = []
    for c in range(_NCORES):
        b, h = divmod(c, 2)
        x = np.asarray(predictions[b, h * _H : (h + 1) * _H], dtype=np.float32)
        y = np.asarray(targets[b], dtype=np.float32)
        u = np.empty((5, _H), np.float32)
        u[0:3] = x.T
        u[3] = (x * x).sum(axis=-1)
        u[4] = 1.0
        v = np.empty((5, _N), np.float32)
        v[0:3] = -2.0 * y.T
        v[3] = 1.0
        v[4] = (y * y).sum(axis=-1)
        u_hi, u_lo = _hi_lo(u)
        v_hi, v_lo = _hi_lo(v)
        uv = np.empty((_K, _H + _N), bf16)
        uv[0:5, :_H] = u_hi
        uv[5:10, :_H] = u_lo
        uv[10:15, :_H] = u_hi
        uv[15:20, :_H] = u_lo
        uv[0:5, _H:] = v_hi
        uv[5:10, _H:] = v_hi
        uv[10:15, _H:] = v_lo
        uv[15:20, _H:] = v_lo
        in_maps.append({"uv": uv})
    return in_maps


def _combine(results):
    loss = 0.0
    for b in range(_B):
        r0, r1 = results[2 * b], results[2 * b + 1]
        cx = np.concatenate(
            [
                np.ascontiguousarray(r0["out_x"].T).ravel(),
                np.ascontiguousarray(r1["out_x"].T).ravel(),
            ]
        )
        cy = np.minimum(
            np.ascontiguousarray(r0["out_y"].T).ravel(),
            np.ascontiguousarray(r1["out_y"].T).ravel(),
        )
        cx = np.maximum(cx, 0.0)
        cy = np.maximum(cy, 0.0)
        loss += cx.mean(dtype=np.float64) + cy.mean(dtype=np.float64)
    loss /= _B
    return np.array(loss, dtype=np.float32)


def kernel(predictions, targets):
    nc = _get_nc()
    in_maps = _make_in_maps(predictions, targets)
    try:
        from concourse.bass_utils import run_bass_kernel_spmd

        res = run_bass_kernel_spmd(nc, in_maps, core_ids=list(range(_NCORES)))
        results = res.results
    except ModuleNotFoundError:
        # axon NTFF trace hook unavailable (BASS_TRACE set in env) — run
        # the execute step directly via PJRT.
        from concourse import bass2jax

        results = bass2jax.run_bass_via_pjrt(nc, in_maps, n_cores=_NCORES)
    return _combine(results)
